# revision 6
# baseline (speedup 1.0000x reference)
"""PointPillar NMS pre-selection kernel for 8x Trainium2 NeuronCores.

Computes, for cls_preds (8M, 3) and box_preds (8M, 7):
    scores = max(cls_preds, axis=-1)
    top_scores, idx = top_k(scores, 4096)   (jax tie-break: score desc, idx asc)
    returns (top_scores, box_preds[idx], argmax(cls_preds[idx])+1)

Strategy: data-parallel over the box dimension (distributed top-k reduction).
Each of the 8 cores DMA-streams its 1M-row cls shard and reduces every block of
16 rows to the block's score maximum (one fused VectorE max-reduce over 48
contiguous floats per block — the row max over 3 classes and the 16-row block
max in a single instruction). That is the entire device program: it reads the
full 96MB of cls data at DMA roofline and emits 8 x 62592 block maxima.

The host merge is exact: every element of the global top-4096 lives in a block
whose max is >= the 4096th-largest block maximum, so gathering the top-4096
blocks (ties included) is a provably sufficient candidate set. The host
rescores those ~4096 blocks (~65K rows), runs the exact jax-equivalent
tie-broken top-k, and gathers boxes/labels for the final 4096 indices only.
box_preds never touches the device.
"""

import numpy as np

N = 8_000_000
NCLS = 3
BOX_DIM = 7
K = 4096
NCORES = 8
N_CORE = N // NCORES      # 1,000,000 rows per core

BLK = 16                  # rows per block-max
ROWS_P = 7824             # rows per partition (128 * 7824 = 1,001,472 >= 1e6)
BPP = ROWS_P // BLK       # 489 blocks per partition
N_PAD = 128 * ROWS_P      # padded rows per core
# chunk widths in blocks per partition (sum = BPP). Small first chunk lets the
# VectorE reduce start as soon as possible; small last chunks shrink the
# post-DMA tail (the final reduce is the only compute after the last byte
# lands). Middle chunks are ~1.2MB for DMA efficiency.
CHUNKS = [13, 52, 52, 52, 52, 52, 52, 52, 52, 36, 12, 12]
assert sum(CHUNKS) == BPP
PAD_VAL = -3.0e38

_CACHE = {}


def _build_program():
    import concourse.bacc as bacc
    import concourse.mybir as mybir
    import concourse.tile as tile

    f32 = mybir.dt.float32

    nc = bacc.Bacc("TRN2", target_bir_lowering=False, debug=False,
                   enable_asserts=False)
    cls_in = nc.dram_tensor("cls", [N_PAD * NCLS], f32, kind="ExternalInput")
    bmax_out = nc.dram_tensor("bmax", [128 * BPP], f32, kind="ExternalOutput")

    bmax_sb = nc.alloc_sbuf_tensor("bmax_sb", [128, BPP], f32)

    with tile.TileContext(nc) as tc:
        with tc.tile_pool(name="raw", bufs=6) as rawp:
            src = cls_in.ap().rearrange("(p f) -> p f", p=128)  # [128, 23472]
            dst = bmax_out.ap().rearrange("(p f) -> p f", p=128)  # [128, BPP]
            bo = 0
            for w in CHUNKS:
                fw = w * BLK * NCLS  # floats per partition this chunk
                raw = rawp.tile([128, fw], f32, name=f"raw{bo}", tag="raw")
                nc.sync.dma_start(
                    out=raw[:, :], in_=src[:, bo * BLK * NCLS : bo * BLK * NCLS + fw]
                )
                nc.vector.tensor_reduce(
                    out=bmax_sb.ap()[:, bo : bo + w],
                    in_=raw[:, :].rearrange("p (a b) -> p a b", b=BLK * NCLS),
                    axis=mybir.AxisListType.X,
                    op=mybir.AluOpType.max,
                )
                # stream each chunk's block maxima out immediately so only the
                # final (small) chunk's reduce+copy sits after the last DMA
                nc.sync.dma_start(out=dst[:, bo : bo + w],
                                  in_=bmax_sb.ap()[:, bo : bo + w])
                bo += w

    nc.compile()
    return nc


def _get_program():
    if "nc" not in _CACHE:
        _CACHE["nc"] = _build_program()
    return _CACHE["nc"]


def _make_in_maps(cls_preds: np.ndarray) -> list[dict]:
    in_maps = []
    for c in range(NCORES):
        shard = np.full((N_PAD, NCLS), PAD_VAL, dtype=np.float32)
        shard[:N_CORE] = cls_preds[c * N_CORE : (c + 1) * N_CORE]
        in_maps.append({"cls": shard.reshape(-1)})
    return in_maps


def _merge_host(results: list[dict], cls_preds: np.ndarray):
    """Exact top-K from per-core block maxima.

    Soundness: let v* be the K-th largest block max. The top-K block maxima
    are K distinct elements each >= v*, so the K-th largest element t* >= v*.
    Any element >= t* lies in a block whose max >= t* >= v*, i.e. in the
    selected set {blocks : bmax >= v*}.
    """
    bm = np.stack([r["bmax"] for r in results])  # [8, 128*BPP]
    flat = bm.reshape(-1)
    part = np.argpartition(-flat, K - 1)
    vstar = flat[part[K - 1]]
    sel = np.flatnonzero(flat >= vstar)  # block ids, ties included

    # decode block id -> 16 global row indices
    c, rem = np.divmod(sel, 128 * BPP)
    p, b = np.divmod(rem, BPP)
    r0 = p.astype(np.int64) * ROWS_P + b.astype(np.int64) * BLK
    rows = (c.astype(np.int64) * N_CORE)[:, None] + r0[:, None] + np.arange(BLK)
    valid_block = r0 + BLK <= N_CORE  # pad blocks are never selected, but be safe
    rows = rows[valid_block].reshape(-1)

    scores = cls_preds[rows].max(axis=1)
    order = np.lexsort((rows, -scores.astype(np.float64)))[:K]
    return scores[order], rows[order]


def kernel(cls_preds: np.ndarray, box_preds: np.ndarray):
    from concourse import bass_utils

    cls_preds = np.asarray(cls_preds, dtype=np.float32)
    box_preds = np.asarray(box_preds, dtype=np.float32)

    nc = _get_program()
    in_maps = _make_in_maps(cls_preds)
    res = bass_utils.run_bass_kernel_spmd(nc, in_maps, core_ids=list(range(NCORES)))

    top_scores, top_idx = _merge_host(res.results, cls_preds)

    top_boxes = box_preds[top_idx]
    top_labels = (cls_preds[top_idx].argmax(axis=1) + 1).astype(np.int32)
    return top_scores.astype(np.float32), top_boxes, top_labels


# revision 8
# speedup vs baseline: 1.1385x; 1.1385x over previous
"""PointPillar NMS pre-selection kernel for 8x Trainium2 NeuronCores.

Computes, for cls_preds (8M, 3) and box_preds (8M, 7):
    scores = max(cls_preds, axis=-1)
    top_scores, idx = top_k(scores, 4096)   (jax tie-break: score desc, idx asc)
    returns (top_scores, box_preds[idx], argmax(cls_preds[idx])+1)

Strategy: data-parallel over the box dimension (distributed top-k reduction).
Each of the 8 cores DMA-streams its 1M-row cls shard and reduces every block of
16 rows to the block's score maximum (one fused VectorE max-reduce over 48
contiguous floats per block — the row max over 3 classes and the 16-row block
max in a single instruction). That is the entire device program: it reads the
full 96MB of cls data at DMA roofline and emits 8 x 62592 block maxima.

The host merge is exact: every element of the global top-4096 lives in a block
whose max is >= the 4096th-largest block maximum, so gathering the top-4096
blocks (ties included) is a provably sufficient candidate set. The host
rescores those ~4096 blocks (~65K rows), runs the exact jax-equivalent
tie-broken top-k, and gathers boxes/labels for the final 4096 indices only.
box_preds never touches the device.
"""

import numpy as np

N = 8_000_000
NCLS = 3
BOX_DIM = 7
K = 4096
NCORES = 8
N_CORE = N // NCORES      # 1,000,000 rows per core

BLK = 16                  # rows per block-max
ROWS_P = 7824             # rows per partition (128 * 7824 = 1,001,472 >= 1e6)
BPP = ROWS_P // BLK       # 489 blocks per partition
N_PAD = 128 * ROWS_P      # padded rows per core
# chunk widths in blocks per partition (sum = BPP). Small first chunk lets the
# VectorE reduce start as soon as possible; small last chunks shrink the
# post-DMA tail (the final reduce is the only compute after the last byte
# lands). Middle chunks are ~1.2MB for DMA efficiency.
CHUNKS = [13, 52, 52, 52, 52, 52, 52, 52, 52, 48, 12]
assert sum(CHUNKS) == BPP
PAD_VAL = -3.0e38

_CACHE = {}


def _build_program():
    import concourse.bacc as bacc
    import concourse.mybir as mybir
    import concourse.tile as tile

    f32 = mybir.dt.float32

    nc = bacc.Bacc("TRN2", target_bir_lowering=False, debug=False,
                   enable_asserts=False)
    cls_in = nc.dram_tensor("cls", [N_PAD * NCLS], f32, kind="ExternalInput")
    bmax_out = nc.dram_tensor("bmax", [128 * BPP], f32, kind="ExternalOutput")

    bmax_sb = nc.alloc_sbuf_tensor("bmax_sb", [128, BPP], f32)

    with tile.TileContext(nc) as tc:
        with tc.tile_pool(name="raw", bufs=8) as rawp:
            src = cls_in.ap().rearrange("(p f) -> p f", p=128)  # [128, 23472]
            dst = bmax_out.ap().rearrange("(p f) -> p f", p=128)  # [128, BPP]
            bo = 0
            for ci, w in enumerate(CHUNKS):
                fw = w * BLK * NCLS  # floats per partition this chunk
                raw = rawp.tile([128, fw], f32, name=f"raw{bo}", tag="raw")
                # input DMAs own the Sync engine's DGE; the two output DMAs go
                # through the Scalar engine's DGE so they never delay the
                # input stream
                nc.sync.dma_start(
                    out=raw[:, :], in_=src[:, bo * BLK * NCLS : bo * BLK * NCLS + fw]
                )
                nc.vector.tensor_reduce(
                    out=bmax_sb.ap()[:, bo : bo + w],
                    in_=raw[:, :].rearrange("p (a b) -> p a b", b=BLK * NCLS),
                    axis=mybir.AxisListType.X,
                    op=mybir.AluOpType.max,
                )
                bo += w
                if ci == len(CHUNKS) - 2:
                    # everything except the last small chunk, overlapped with
                    # the final DMA+reduce
                    nc.scalar.dma_start(out=dst[:, :bo], in_=bmax_sb.ap()[:, :bo])
            nc.scalar.dma_start(out=dst[:, bo - CHUNKS[-1] :],
                                in_=bmax_sb.ap()[:, bo - CHUNKS[-1] :])

    nc.compile()
    return nc


def _get_program():
    if "nc" not in _CACHE:
        _CACHE["nc"] = _build_program()
    return _CACHE["nc"]


def _make_in_maps(cls_preds: np.ndarray) -> list[dict]:
    in_maps = []
    for c in range(NCORES):
        shard = np.full((N_PAD, NCLS), PAD_VAL, dtype=np.float32)
        shard[:N_CORE] = cls_preds[c * N_CORE : (c + 1) * N_CORE]
        in_maps.append({"cls": shard.reshape(-1)})
    return in_maps


def _merge_host(results: list[dict], cls_preds: np.ndarray):
    """Exact top-K from per-core block maxima.

    Soundness: let v* be the K-th largest block max. The top-K block maxima
    are K distinct elements each >= v*, so the K-th largest element t* >= v*.
    Any element >= t* lies in a block whose max >= t* >= v*, i.e. in the
    selected set {blocks : bmax >= v*}.
    """
    bm = np.stack([r["bmax"] for r in results])  # [8, 128*BPP]
    flat = bm.reshape(-1)
    part = np.argpartition(-flat, K - 1)
    vstar = flat[part[K - 1]]
    sel = np.flatnonzero(flat >= vstar)  # block ids, ties included

    # decode block id -> 16 global row indices
    c, rem = np.divmod(sel, 128 * BPP)
    p, b = np.divmod(rem, BPP)
    r0 = p.astype(np.int64) * ROWS_P + b.astype(np.int64) * BLK
    rows = (c.astype(np.int64) * N_CORE)[:, None] + r0[:, None] + np.arange(BLK)
    valid_block = r0 + BLK <= N_CORE  # pad blocks are never selected, but be safe
    rows = rows[valid_block].reshape(-1)

    scores = cls_preds[rows].max(axis=1)
    order = np.lexsort((rows, -scores.astype(np.float64)))[:K]
    return scores[order], rows[order]


def kernel(cls_preds: np.ndarray, box_preds: np.ndarray):
    from concourse import bass_utils

    cls_preds = np.asarray(cls_preds, dtype=np.float32)
    box_preds = np.asarray(box_preds, dtype=np.float32)

    nc = _get_program()
    in_maps = _make_in_maps(cls_preds)
    res = bass_utils.run_bass_kernel_spmd(nc, in_maps, core_ids=list(range(NCORES)))

    top_scores, top_idx = _merge_host(res.results, cls_preds)

    top_boxes = box_preds[top_idx]
    top_labels = (cls_preds[top_idx].argmax(axis=1) + 1).astype(np.int32)
    return top_scores.astype(np.float32), top_boxes, top_labels


# revision 12
# speedup vs baseline: 1.2608x; 1.1074x over previous
"""PointPillar NMS pre-selection kernel for 8x Trainium2 NeuronCores.

Computes, for cls_preds (8M, 3) and box_preds (8M, 7):
    scores = max(cls_preds, axis=-1)
    top_scores, idx = top_k(scores, 4096)   (jax tie-break: score desc, idx asc)
    returns (top_scores, box_preds[idx], argmax(cls_preds[idx])+1)

Strategy: data-parallel over the box dimension (distributed top-k reduction).
Each of the 8 cores DMA-streams its 1M-row cls shard and reduces every block of
16 rows to the block's score maximum (one fused VectorE max-reduce over 48
contiguous floats per block — the row max over 3 classes and the 16-row block
max in a single instruction). That is the entire device program: it reads the
full 96MB of cls data at DMA roofline and emits 8 x 62592 block maxima.

The host merge is exact: every element of the global top-4096 lives in a block
whose max is >= the 4096th-largest block maximum, so gathering the top-4096
blocks (ties included) is a provably sufficient candidate set. The host
rescores those ~4096 blocks (~65K rows), runs the exact jax-equivalent
tie-broken top-k, and gathers boxes/labels for the final 4096 indices only.
box_preds never touches the device.
"""

import numpy as np

N = 8_000_000
NCLS = 3
BOX_DIM = 7
K = 4096
NCORES = 8
N_CORE = N // NCORES      # 1,000,000 rows per core

BLK = 16                  # rows per block-max
ROWS_P = 7824             # rows per partition (128 * 7824 = 1,001,472 >= 1e6)
BPP = ROWS_P // BLK       # 489 blocks per partition
N_PAD = 128 * ROWS_P      # padded rows per core
# chunk widths in blocks per partition (sum = BPP). Small first chunk lets the
# VectorE reduce start as soon as possible; small last chunks shrink the
# post-DMA tail (the final reduce is the only compute after the last byte
# lands). Middle chunks are ~1.2MB for DMA efficiency.
CHUNKS = [13, 52, 52, 52, 52, 52, 52, 52, 52, 48, 12]
assert sum(CHUNKS) == BPP
PAD_VAL = -3.0e38

_CACHE = {}


NBUF = 8


def _build_program():
    """Raw (non-Tile) pipeline with manual semaphores:

      Sync engine:   issues the 11 input DMA chunks (with buffer-reuse
                     backpressure against the VectorE reduce counter)
      Vector engine: one windowed max-reduce per chunk into bmax_sb
      Scalar engine: final DMA of the block maxima to DRAM, then waits for
                     its completion so the NEFF can't finish early

    Avoids Tile's scheduling barriers/drains (~7us of a 46us kernel).
    """
    import concourse.bacc as bacc
    import concourse.mybir as mybir

    f32 = mybir.dt.float32

    nc = bacc.Bacc("TRN2", target_bir_lowering=False, debug=False,
                   enable_asserts=False)
    cls_in = nc.dram_tensor("cls", [N_PAD * NCLS], f32, kind="ExternalInput")
    bmax_out = nc.dram_tensor("bmax", [128 * BPP], f32, kind="ExternalOutput")

    bmax_sb = nc.alloc_sbuf_tensor("bmax_sb", [128, BPP], f32)
    wmax = max(CHUNKS) * BLK * NCLS
    raws = [
        nc.alloc_sbuf_tensor(f"raw{i}", [128, wmax], f32) for i in range(NBUF)
    ]

    # one DMA-completion semaphore per buffer: chunk DMAs can complete out of
    # order across hardware queues, so a single shared counter would be racy
    dma_sems = [nc.alloc_semaphore(f"dma_sem{i}") for i in range(NBUF)]
    red_sem = nc.alloc_semaphore("red_sem")
    out_sem = nc.alloc_semaphore("out_sem")

    src = cls_in.ap().rearrange("(p f) -> p f", p=128)  # [128, 23472]
    dst = bmax_out.ap().rearrange("(p f) -> p f", p=128)  # [128, BPP]

    offs = [0]
    for w in CHUNKS:
        offs.append(offs[-1] + w)

    with nc.Block() as blk:

        @blk.sync
        def _(sync):
            for i, w in enumerate(CHUNKS):
                if i >= NBUF:
                    sync.wait_ge(red_sem, i - NBUF + 1)
                fw = w * BLK * NCLS
                f0 = offs[i] * BLK * NCLS
                sync.dma_start(
                    out=raws[i % NBUF].ap()[:, :fw], in_=src[:, f0 : f0 + fw]
                ).then_inc(dma_sems[i % NBUF], 16)

        @blk.vector
        def _(vector):
            for i, w in enumerate(CHUNKS):
                vector.wait_ge(dma_sems[i % NBUF], 16 * (i // NBUF + 1))
                fw = w * BLK * NCLS
                vector.tensor_reduce(
                    out=bmax_sb.ap()[:, offs[i] : offs[i] + w],
                    in_=raws[i % NBUF].ap()[:, :fw].rearrange(
                        "p (a b) -> p a b", b=BLK * NCLS
                    ),
                    axis=mybir.AxisListType.X,
                    op=mybir.AluOpType.max,
                ).then_inc(red_sem, 1)

        @blk.scalar
        def _(scalar):
            scalar.wait_ge(red_sem, len(CHUNKS))
            scalar.dma_start(out=dst, in_=bmax_sb.ap()).then_inc(out_sem, 16)
            scalar.wait_ge(out_sem, 16)

    nc.compile()
    return nc


def _get_program():
    if "nc" not in _CACHE:
        _CACHE["nc"] = _build_program()
    return _CACHE["nc"]


def _make_in_maps(cls_preds: np.ndarray) -> list[dict]:
    in_maps = []
    for c in range(NCORES):
        shard = np.full((N_PAD, NCLS), PAD_VAL, dtype=np.float32)
        shard[:N_CORE] = cls_preds[c * N_CORE : (c + 1) * N_CORE]
        in_maps.append({"cls": shard.reshape(-1)})
    return in_maps


def _merge_host(results: list[dict], cls_preds: np.ndarray):
    """Exact top-K from per-core block maxima.

    Soundness: let v* be the K-th largest block max. The top-K block maxima
    are K distinct elements each >= v*, so the K-th largest element t* >= v*.
    Any element >= t* lies in a block whose max >= t* >= v*, i.e. in the
    selected set {blocks : bmax >= v*}.
    """
    bm = np.stack([r["bmax"] for r in results])  # [8, 128*BPP]
    flat = bm.reshape(-1)
    part = np.argpartition(-flat, K - 1)
    vstar = flat[part[K - 1]]
    sel = np.flatnonzero(flat >= vstar)  # block ids, ties included

    # decode block id -> 16 global row indices
    c, rem = np.divmod(sel, 128 * BPP)
    p, b = np.divmod(rem, BPP)
    r0 = p.astype(np.int64) * ROWS_P + b.astype(np.int64) * BLK
    rows = (c.astype(np.int64) * N_CORE)[:, None] + r0[:, None] + np.arange(BLK)
    valid_block = r0 + BLK <= N_CORE  # pad blocks are never selected, but be safe
    rows = rows[valid_block].reshape(-1)

    scores = cls_preds[rows].max(axis=1)
    order = np.lexsort((rows, -scores.astype(np.float64)))[:K]
    return scores[order], rows[order]


def kernel(cls_preds: np.ndarray, box_preds: np.ndarray):
    from concourse import bass_utils

    cls_preds = np.asarray(cls_preds, dtype=np.float32)
    box_preds = np.asarray(box_preds, dtype=np.float32)

    nc = _get_program()
    in_maps = _make_in_maps(cls_preds)
    res = bass_utils.run_bass_kernel_spmd(nc, in_maps, core_ids=list(range(NCORES)))

    top_scores, top_idx = _merge_host(res.results, cls_preds)

    top_boxes = box_preds[top_idx]
    top_labels = (cls_preds[top_idx].argmax(axis=1) + 1).astype(np.int32)
    return top_scores.astype(np.float32), top_boxes, top_labels
